# revision 34
# baseline (speedup 1.0000x reference)
"""DigitCaps dynamic-routing kernel for 8 Trainium2 NeuronCores.

Sharding: J (num_capsule=32) split 8 ways -> 4 capsules per core, batch
replicated. W is SBUF-resident in its natural layout for the i-contraction
GEMMs; the transposed layout is streamed for the p-contraction routing
matmuls. The routing softmax over J uses a cross-core AllReduce of
per-(b,i) partial exp sums; a renormalization-invariance trick keeps a
single running tensor F (= c, up to a shared normalizer) instead of exp(b).

Per core (j = 4 local capsules, B=64, I=2048, Q=16, P=32):
  hat[b,j,i,p] = sum_q x[b,i,q] W[j,i,p,q]       (never materialized)
  v1 = squash(S/32),  S = sum_{i,q} x W          (c1 uniform)
  Delta_k[b,j,i] = sum_q x[b,i,q] * (Wt^T vbd_k)[b,j,(i,q)]
  F <- F * exp(Delta);  Z = AllReduce_j(sum_j F)  (4 i-chunks, pipelined)
  v_k = squash(sum_{i,q} (F/Z x) W)
  out = v3

Pipelining structure vs the naive version:
  - b-pass PSUM ping-pong (1024-col chunks, 3 bufs) so PE streams ahead
    while ScalarE evacuates and DVE multiplies/reduces.
  - wt/xt columns ordered (g, iw, q) so the q-reduction is a single
    DVE tensor_reduce(axis=X) per chunk.
  - AllReduce split into 4 i-chunks issued at 1/4 intervals of the
    b-pass; v-pass consumes chunk-by-chunk.
  - Resident inputs DMA'd in ich-chunks overlapped with the S-pass.
  - GpSimd (Pool) carries F-update, Z partial sums, and F-normalize.
"""

import numpy as np
import ml_dtypes

import concourse.bacc as bacc
import concourse.mybir as mybir
import concourse.tile as tile
from concourse.bass_utils import run_bass_kernel_spmd
from concourse.masks import make_identity

BF16 = mybir.dt.bfloat16
F32 = mybir.dt.float32
NP_BF16 = ml_dtypes.bfloat16

N_CORES = 8
B = 64
I = 2048
Q = 16
J = 32
P = 32
JL = J // N_CORES
ICH = I // 128
EPS = 1e-7
AF = mybir.ActivationFunctionType
NCHUNK = 8              # AllReduce chunks, uneven: big early (slack to
CHUNKS = [3, 3, 2, 2, 2, 2, 1, 1]   # hide), tiny last (short tail latency)
CH_OFF = [0]
for _c in CHUNKS:
    CH_OFF.append(CH_OFF[-1] + _c)
assert CH_OFF[-1] == ICH
# g's whose pair-1 q-fold runs on GpSimd (never at an AR chunk boundary,
# so the Pool queue cannot delay a z-sum -> AllReduce issue)
POOL_G = {0, 3, 6, 8, 10, 12}

_CACHED = {}


def _squash(nc, small, v_sb, eps_ap):
    """In-place squash over p of v_sb [64, JL*P] fp32 (free = (j, p))."""
    sq = small.tile([B, JL * P], F32, tag="sq")
    nc.vector.tensor_mul(sq[:], v_sb[:], v_sb[:])
    s2 = small.tile([B, JL], F32, tag="s2")
    nc.vector.tensor_reduce(
        s2[:], sq.rearrange("b (j p) -> b j p", j=JL)[:],
        mybir.AxisListType.X, mybir.AluOpType.add,
    )
    rt = small.tile([B, JL], F32, tag="rt")
    nc.scalar.activation(rt[:], s2[:], AF.Sqrt, bias=eps_ap[:B, :])
    den = small.tile([B, JL], F32, tag="den")
    nc.vector.tensor_mul(den[:], s2[:], rt[:])
    nc.vector.tensor_add(den[:], den[:], rt[:])
    rec = small.tile([B, JL], F32, tag="rec")
    nc.vector.reciprocal(rec[:], den[:])
    scale = small.tile([B, JL], F32, tag="scale")
    nc.vector.tensor_mul(scale[:], s2[:], rec[:])
    vv = v_sb.rearrange("b (j p) -> b j p", j=JL)
    sc_b = scale.unsqueeze(2).broadcast_to([B, JL, P])
    nc.vector.tensor_mul(vv[:], vv[:], sc_b[:])


def _build_vbd(nc, small, psum_t, v_sb, identity):
    """v_sb [64, (j,p)] fp32 -> two block-diag bf16 lhsT [128, (jj 2, b 64)]."""
    vt_ps = psum_t.tile([128, B], F32, tag="vt_ps")
    nc.tensor.transpose(vt_ps[:], v_sb[:], identity[:B, :B])
    vt = small.tile([128, B], F32, tag="vt")
    nc.scalar.copy(vt[:], vt_ps[:])  # [(j,p), b]
    vbds = []
    for pair in range(2):
        vbd = small.tile([128, 2 * B], BF16, tag=f"vbd{pair}")
        nc.vector.memset(vbd[:], 0.0)
        for jj in range(2):
            j = pair * 2 + jj
            nc.vector.tensor_copy(
                vbd[j * P:(j + 1) * P, jj * B:(jj + 1) * B],
                vt[j * P:(j + 1) * P, :],
            )
        vbds.append(vbd)
    return vbds


def _vT_to_v(nc, small, ps_vt, vT_ps, identity, scale=None):
    """vT psum [128 (j,p), 64 b] -> v_sb [64, (j,p)] fp32 via evac+transpose."""
    vT = small.tile([128, B], F32, tag="vTe")
    if scale is None:
        nc.scalar.copy(vT[:], vT_ps[:])
    else:
        nc.scalar.mul(vT[:], vT_ps[:], scale)
    v_ps = ps_vt.tile([B, 128], F32, tag="v_ps2")
    nc.tensor.transpose(v_ps[:], vT[:], identity[:])
    v_sb = small.tile([B, JL * P], F32, tag="v")
    nc.scalar.copy(v_sb[:], v_ps[:])
    return v_sb


def build_kernel():
    if "nc" in _CACHED:
        return _CACHED["nc"]
    nc = bacc.Bacc(
        "TRN2", target_bir_lowering=False, debug=False, num_devices=N_CORES
    )
    wn_d = nc.dram_tensor("wn", [128, ICH * Q * JL * P], BF16, kind="ExternalInput")
    wt_d = nc.dram_tensor("wt", [128, I * Q], BF16, kind="ExternalInput")
    xq_d = nc.dram_tensor("xq", [128, ICH * Q * B], BF16, kind="ExternalInput")
    xt_d = nc.dram_tensor("xt", [128, I * Q], BF16, kind="ExternalInput")
    out_d = nc.dram_tensor("o", [B, JL * P], F32, kind="ExternalOutput")

    with tile.TileContext(nc) as tc:
        with (
            tc.tile_pool(name="big", bufs=1) as big,
            tc.tile_pool(name="wts", bufs=2) as wts,
            tc.tile_pool(name="evac", bufs=2) as evac,
            tc.tile_pool(name="ustr", bufs=2) as ustr,
            tc.tile_pool(name="small", bufs=1) as small,
            tc.tile_pool(name="ytile", bufs=2) as ytile,
            tc.tile_pool(name="dram", bufs=8, space="DRAM") as dram,
        ):
            # wt_s prefetch helper; first two windows fetched before the
            # bulk resident loads so iteration-0's b-pass can start early
            wt_tiles = {}

            def fetch_wt(it, g):
                t = wts.tile(
                    [128, 128 * Q], BF16, tag="wt_s", name=f"wt{it}_{g}"
                )
                nc.sync.dma_start(t[:], wt_d[:, g * 128 * Q:(g + 1) * 128 * Q])
                wt_tiles[(it, g)] = t

            fetch_wt(0, 0)
            fetch_wt(0, 1)

            # ---- resident loads (chunked by ich group) ---------------
            xq = big.tile([128, ICH * Q * B], BF16, tag="xq")        # 32K/part
            xqv = xq.rearrange("k (ich q b) -> k ich q b", ich=ICH, q=Q)
            wn = big.tile([128, ICH * Q * JL * P], BF16, tag="wn")   # 64K/part
            wnv = wn.rearrange("k (ich q j p) -> k ich q j p", ich=ICH, q=Q, j=JL)
            XQC = ICH * Q * B // 4
            WNC = ICH * Q * JL * P // 4
            for k in range(4):
                nc.sync.dma_start(
                    xq[:, k * XQC:(k + 1) * XQC], xq_d[:, k * XQC:(k + 1) * XQC]
                )
                nc.sync.dma_start(
                    wn[:, k * WNC:(k + 1) * WNC], wn_d[:, k * WNC:(k + 1) * WNC]
                )
            xt = big.tile([128, I * Q], BF16, tag="xt")              # 64K/part
            XTC = I * Q // 4
            for k in range(4):
                nc.sync.dma_start(
                    xt[:, k * XTC:(k + 1) * XTC],
                    xt_d[:, k * XTC:(k + 1) * XTC],
                )

            identity = big.tile([128, 128], F32, tag="ident")
            make_identity(nc, identity[:])
            identb = big.tile([128, 128], BF16, tag="identb")
            make_identity(nc, identb[:])
            eps_t = big.tile([128, 1], F32, tag="eps")
            nc.vector.memset(eps_t[:], EPS)

            # F[iw, (ich, j, b)] bf16: running c (up to per-(b,i) normalizer)
            f_sb = big.tile([128, ICH * JL * B], BF16, tag="f")      # 8K/part
            f_v = f_sb.rearrange("k (ich j b) -> k ich j b", ich=ICH, j=JL)

            # warmup collective to absorb core-start skew
            wu_s = small.tile([128, 8], F32, tag="wu")
            nc.gpsimd.memset(wu_s[:], 0.0)
            wu_i = dram.tile([128, 8], F32, tag="wu_i")
            wu_o = dram.tile([128, 8], F32, tag="wu_o")
            nc.gpsimd.dma_start(wu_i[:], wu_s[:])
            nc.gpsimd.collective_compute(
                "AllReduce", mybir.AluOpType.add,
                replica_groups=[list(range(N_CORES))],
                ins=[wu_i.opt()], outs=[wu_o.opt()],
            )

            # ---- S-pass: vT[(j,p), b] = sum_{i,q} W x ---------------
            with tc.tile_pool(name="ps_s", bufs=1, space="PSUM") as ps_s, \
                 tc.tile_pool(name="ps_st", bufs=1, space="PSUM") as ps_st:
                s_ps = ps_s.tile([128, B], F32, tag="s_ps")
                n_mm = ICH * Q
                k = 0
                for ich in range(ICH):
                    for q in range(Q):
                        nc.tensor.matmul(
                            s_ps[:],
                            wnv[:, ich, q, :, :],       # lhsT [128, (j p)]
                            xqv[:, ich, q, :],          # rhs  [128, 64]
                            start=(k == 0), stop=(k == n_mm - 1),
                        )
                        k += 1
                v_sb = _vT_to_v(nc, small, ps_st, s_ps, identity, scale=1.0 / J)
                _squash(nc, small, v_sb, eps_t)
                vbds = _build_vbd(nc, small, ps_st, v_sb, identity)

            # ---- 2 routing iterations -------------------------------
            for it in range(2):
                first = it == 0

                def v_chunk(ck, it=it):
                    """Emit v-pass work for AllReduce chunk ck."""
                    cl = CHUNKS[ck]
                    sl = slice(CH_OFF[ck], CH_OFF[ck + 1])
                    zh = small.tile(
                        [128, cl * B], BF16, tag=f"z{ck}",
                        name=f"z{it}_{ck}",
                    )
                    # trigger from Act: SP would stall the wt_s prefetch
                    # stream behind this AR-gated wait
                    nc.scalar.dma_start(zh[:], cc_out[ck][:])
                    zr = small.tile(
                        [128, cl * B], F32, tag=f"zr{ck}",
                        name=f"zr{it}_{ck}",
                    )
                    nc.vector.reciprocal(zr[:], zh[:])
                    zrv = zr.rearrange("k (ic b) -> k ic b", ic=cl)
                    zrb = zrv.unsqueeze(2).broadcast_to(
                        [128, cl, JL, B]
                    )
                    nc.gpsimd.tensor_mul(
                        f_v[:, sl, :, :], f_v[:, sl, :, :], zrb[:]
                    )
                    for ich in range(CH_OFF[ck], CH_OFF[ck + 1]):
                        for pr in range(2):
                            y = ytile.tile(
                                [128, 2 * Q * B], BF16, tag="y",
                                name=f"y{it}_{ich}_{pr}",
                            )
                            yv = y.rearrange(
                                "k (j q b) -> k j q b", j=2, q=Q
                            )
                            cb = (
                                f_v[:, ich, 2 * pr:2 * pr + 2, :]
                                .unsqueeze(2).broadcast_to([128, 2, Q, B])
                            )
                            xb = (
                                xqv[:, ich, :, :]
                                .unsqueeze(1).broadcast_to([128, 2, Q, B])
                            )
                            nc.vector.tensor_mul(yv[:], xb[:], cb[:])
                            for q in range(Q):
                                for jj in range(2):
                                    j = 2 * pr + jj
                                    nc.tensor.matmul(
                                        vT_ps[j * P:(j + 1) * P, :],
                                        wnv[:, ich, q, j, :],
                                        yv[:, jj, q, :],
                                        start=(ich == 0 and q == 0),
                                        stop=(ich == ICH - 1 and q == Q - 1),
                                        tile_position=(0, j * P),
                                    )

                # b-pass: wt cols per g are (iw 128, q 16); per half h the
                # 1024-col window is iw 64..(64h+63) x q 16.
                cc_out = [None] * NCHUNK
                with tc.tile_pool(name=f"ps_v{it}", bufs=1, space="PSUM") as ps_v:
                  vT_ps = ps_v.tile([128, B], F32, tag="vT_ps")
                  with tc.tile_pool(name=f"ps_b{it}", bufs=2, space="PSUM") as ps_b, \
                       tc.tile_pool(name=f"ps_bt{it}", bufs=2, space="PSUM") as ps_bt:
                    for g in range(ICH):
                        if (it, g) not in wt_tiles:
                            fetch_wt(it, g)
                        wt_s = wt_tiles.pop((it, g))
                        if g + 2 < ICH:
                            fetch_wt(it, g + 2)
                        for pair in range(2):
                            dwin = ustr.tile(
                                [128, 128], BF16, tag="dwin",
                                name=f"dwin{it}_{g}_{pair}",
                            )
                            for h in range(2):
                                t_ps = ps_b.tile(
                                    [128, 1024], F32, tag="t_ps",
                                    name=f"t_ps{it}_{g}_{pair}_{h}",
                                )
                                for m in range(2):
                                    nc.tensor.matmul(
                                        t_ps[:, m * 512:(m + 1) * 512],
                                        vbds[pair][:],
                                        wt_s[:, h * 1024 + m * 512:
                                             h * 1024 + (m + 1) * 512],
                                        start=True, stop=True,
                                    )
                                t_sb = evac.tile(
                                    [128, 1024], BF16, tag="t_sb",
                                    name=f"t_sb{it}_{g}_{pair}_{h}",
                                )
                                nc.scalar.copy(t_sb[:], t_ps[:])
                                u = ustr.tile(
                                    [128, 1024], BF16, tag="u",
                                    name=f"u{it}_{g}_{pair}_{h}",
                                )
                                xoff = g * 2048 + h * 1024
                                nc.vector.tensor_mul(
                                    u[:], t_sb[:], xt[:, xoff:xoff + 1024]
                                )
                                uq = u.rearrange("k (iw q) -> k iw q", q=Q)
                                if pair == 1 and g in POOL_G:
                                    # q-fold on GpSimd: frees ~2.4us of DVE
                                    # per selected g; strided halving adds
                                    w = Q
                                    while w > 2:
                                        hw = w // 2
                                        nc.gpsimd.tensor_add(
                                            uq[:, :, 0:hw], uq[:, :, 0:hw],
                                            uq[:, :, hw:w],
                                        )
                                        w = hw
                                    nc.gpsimd.tensor_add(
                                        dwin[:, h * 64:(h + 1) * 64],
                                        uq[:, :, 0], uq[:, :, 1],
                                    )
                                else:
                                    with nc.allow_low_precision(
                                        reason="routing logits tolerate bf16"
                                    ):
                                        nc.vector.tensor_reduce(
                                            dwin[:, h * 64:(h + 1) * 64],
                                            uq[:],
                                            mybir.AxisListType.X,
                                            mybir.AluOpType.add,
                                        )
                            d_ps = ps_bt.tile([128, 128], BF16, tag="d_ps")
                            nc.tensor.transpose(d_ps[:], dwin[:], identb[:])
                            off = (g * JL + pair * 2) * B
                            dst = f_sb[:, off:off + 2 * B]
                            if first:
                                nc.scalar.activation(dst, d_ps[:], AF.Exp)
                            else:
                                ex = ustr.tile([128, 128], BF16, tag="ex")
                                nc.scalar.activation(ex[:], d_ps[:], AF.Exp)
                                nc.gpsimd.tensor_mul(dst, dst, ex[:])
                        if g + 1 in CH_OFF:
                            ck = CH_OFF.index(g + 1) - 1
                            cl = CHUNKS[ck]
                            sl = slice(CH_OFF[ck], CH_OFF[ck + 1])
                            zph = small.tile(
                                [128, cl * B], BF16, tag=f"zp{ck}",
                                name=f"zp{it}_{ck}",
                            )
                            zpv = zph.rearrange("k (ic b) -> k ic b", ic=cl)
                            nc.gpsimd.tensor_add(
                                zpv[:], f_v[:, sl, 0, :], f_v[:, sl, 1, :]
                            )
                            for j in range(2, JL):
                                nc.gpsimd.tensor_add(
                                    zpv[:], zpv[:], f_v[:, sl, j, :]
                                )
                            cc_i = dram.tile(
                                [128, cl * B], BF16, tag=f"cc_i{ck}",
                                name=f"cci{it}_{ck}",
                            )
                            cc_o = dram.tile(
                                [128, cl * B], BF16, tag=f"cc_o{ck}",
                                name=f"cco{it}_{ck}", addr_space="Shared",
                            )
                            nc.gpsimd.dma_start(cc_i[:], zph[:])
                            nc.gpsimd.collective_compute(
                                "AllReduce", mybir.AluOpType.add,
                                replica_groups=[list(range(N_CORES))],
                                ins=[cc_i.opt()], outs=[cc_o.opt()],
                            )
                            cc_out[ck] = cc_o
                            if ck >= 2:
                                v_chunk(ck - 2)
                  # b-pass PSUM pools closed; finish the last two v-chunks
                  v_chunk(NCHUNK - 2)
                  v_chunk(NCHUNK - 1)
                  with tc.tile_pool(
                      name=f"ps_vt{it}", bufs=2, space="PSUM"
                  ) as ps_vt:
                    v_sb = _vT_to_v(nc, small, ps_vt, vT_ps, identity)
                    _squash(nc, small, v_sb, eps_t)
                    if it == 0:
                        vbds = _build_vbd(nc, small, ps_vt, v_sb, identity)
                    else:
                        nc.sync.dma_start(out_d[:], v_sb[:])

    nc.compile()
    _CACHED["nc"] = nc
    return nc


def _prep_inputs(inputs_np, W_np):
    x = np.ascontiguousarray(inputs_np)           # [B, I, Q] f32
    W = np.ascontiguousarray(W_np)                # [J, I, P, Q] f32
    xq = (
        x.reshape(B, ICH, 128, Q).transpose(2, 1, 3, 0)
        .astype(NP_BF16).reshape(128, ICH * Q * B)
    )
    # xt cols ordered (g, iw, q): natural x layout; b-duplication is on-chip
    xt_base = x.astype(NP_BF16).reshape(B, I * Q)
    xt = np.concatenate([xt_base, xt_base], axis=0)
    in_maps = []
    for r in range(N_CORES):
        Wr = W[r * JL:(r + 1) * JL]                       # [4, I, P, Q]
        wn = (
            Wr.reshape(JL, ICH, 128, P, Q).transpose(2, 1, 4, 0, 3)
            .astype(NP_BF16).reshape(128, ICH * Q * JL * P)
        )
        # wt rows (j,p); cols ordered (g, iw, q)
        wt = (
            Wr.reshape(JL, ICH, 128, P, Q)
            .transpose(0, 3, 1, 2, 4)                     # [j, p, g, iw, q]
            .astype(NP_BF16).reshape(128, I * Q)
        )
        in_maps.append(
            {
                "wn": np.ascontiguousarray(wn),
                "wt": np.ascontiguousarray(wt),
                "xq": np.ascontiguousarray(xq),
                "xt": np.ascontiguousarray(xt),
            }
        )
    return in_maps


def kernel(inputs, W, _trace=False):
    nc = build_kernel()
    in_maps = _prep_inputs(np.asarray(inputs), np.asarray(W))
    res = run_bass_kernel_spmd(nc, in_maps, list(range(N_CORES)), trace=_trace)
    out = np.concatenate(
        [res.results[r]["o"].reshape(B, JL, P) for r in range(N_CORES)], axis=1
    )
    if _trace:
        kernel.last_exec_ns = res.exec_time_ns
        kernel.last_results = res
    return out.astype(np.float32)


# revision 36
# speedup vs baseline: 1.0702x; 1.0702x over previous
"""DigitCaps dynamic-routing kernel for 8 Trainium2 NeuronCores.

Sharding: J (num_capsule=32) split 8 ways -> 4 capsules per core, batch
replicated. W is SBUF-resident in its natural layout for the i-contraction
GEMMs; the transposed layout is streamed for the p-contraction routing
matmuls. The routing softmax over J uses a cross-core AllReduce of
per-(b,i) partial exp sums; a renormalization-invariance trick keeps a
single running tensor F (= c, up to a shared normalizer) instead of exp(b).

Per core (j = 4 local capsules, B=64, I=2048, Q=16, P=32):
  hat[b,j,i,p] = sum_q x[b,i,q] W[j,i,p,q]       (never materialized)
  v1 = squash(S/32),  S = sum_{i,q} x W          (c1 uniform)
  Delta_k[b,j,i] = sum_q x[b,i,q] * (Wt^T vbd_k)[b,j,(i,q)]
  F <- F * exp(Delta);  Z = AllReduce_j(sum_j F)  (4 i-chunks, pipelined)
  v_k = squash(sum_{i,q} (F/Z x) W)
  out = v3

Pipelining structure vs the naive version:
  - b-pass PSUM ping-pong (1024-col chunks, 3 bufs) so PE streams ahead
    while ScalarE evacuates and DVE multiplies/reduces.
  - wt/xt columns ordered (g, iw, q) so the q-reduction is a single
    DVE tensor_reduce(axis=X) per chunk.
  - AllReduce split into 4 i-chunks issued at 1/4 intervals of the
    b-pass; v-pass consumes chunk-by-chunk.
  - Resident inputs DMA'd in ich-chunks overlapped with the S-pass.
  - GpSimd (Pool) carries F-update, Z partial sums, and F-normalize.
"""

import numpy as np
import ml_dtypes

import concourse.bacc as bacc
import concourse.mybir as mybir
import concourse.tile as tile
from concourse.bass_utils import run_bass_kernel_spmd
from concourse.masks import make_identity

BF16 = mybir.dt.bfloat16
F32 = mybir.dt.float32
NP_BF16 = ml_dtypes.bfloat16

N_CORES = 8
B = 64
I = 2048
Q = 16
J = 32
P = 32
JL = J // N_CORES
ICH = I // 128
EPS = 1e-7
AF = mybir.ActivationFunctionType
NCHUNK = 8              # AllReduce chunks, uneven: big early (slack to
CHUNKS = [3, 3, 2, 2, 2, 2, 1, 1]   # hide), tiny last (short tail latency)
CH_OFF = [0]
for _c in CHUNKS:
    CH_OFF.append(CH_OFF[-1] + _c)
assert CH_OFF[-1] == ICH

_CACHED = {}


def _squash(nc, small, v_sb, eps_ap):
    """In-place squash over p of v_sb [64, JL*P] fp32 (free = (j, p))."""
    sq = small.tile([B, JL * P], F32, tag="sq")
    nc.vector.tensor_mul(sq[:], v_sb[:], v_sb[:])
    s2 = small.tile([B, JL], F32, tag="s2")
    nc.vector.tensor_reduce(
        s2[:], sq.rearrange("b (j p) -> b j p", j=JL)[:],
        mybir.AxisListType.X, mybir.AluOpType.add,
    )
    rt = small.tile([B, JL], F32, tag="rt")
    nc.scalar.activation(rt[:], s2[:], AF.Sqrt, bias=eps_ap[:B, :])
    den = small.tile([B, JL], F32, tag="den")
    nc.vector.tensor_mul(den[:], s2[:], rt[:])
    nc.vector.tensor_add(den[:], den[:], rt[:])
    rec = small.tile([B, JL], F32, tag="rec")
    nc.vector.reciprocal(rec[:], den[:])
    scale = small.tile([B, JL], F32, tag="scale")
    nc.vector.tensor_mul(scale[:], s2[:], rec[:])
    vv = v_sb.rearrange("b (j p) -> b j p", j=JL)
    sc_b = scale.unsqueeze(2).broadcast_to([B, JL, P])
    nc.vector.tensor_mul(vv[:], vv[:], sc_b[:])


def _build_vbd(nc, small, psum_t, v_sb, identity):
    """v_sb [64, (j,p)] fp32 -> two block-diag bf16 lhsT [128, (jj 2, b 64)]."""
    vt_ps = psum_t.tile([128, B], F32, tag="vt_ps")
    nc.tensor.transpose(vt_ps[:], v_sb[:], identity[:B, :B])
    vt = small.tile([128, B], F32, tag="vt")
    nc.scalar.copy(vt[:], vt_ps[:])  # [(j,p), b]
    vbds = []
    for pair in range(2):
        vbd = small.tile([128, 2 * B], BF16, tag=f"vbd{pair}")
        nc.vector.memset(vbd[:], 0.0)
        for jj in range(2):
            j = pair * 2 + jj
            nc.vector.tensor_copy(
                vbd[j * P:(j + 1) * P, jj * B:(jj + 1) * B],
                vt[j * P:(j + 1) * P, :],
            )
        vbds.append(vbd)
    return vbds


def _vT_to_v(nc, small, ps_vt, vT_ps, identity, scale=None):
    """vT psum [128 (j,p), 64 b] -> v_sb [64, (j,p)] fp32 via evac+transpose."""
    vT = small.tile([128, B], F32, tag="vTe")
    if scale is None:
        nc.scalar.copy(vT[:], vT_ps[:])
    else:
        nc.scalar.mul(vT[:], vT_ps[:], scale)
    v_ps = ps_vt.tile([B, 128], F32, tag="v_ps2")
    nc.tensor.transpose(v_ps[:], vT[:], identity[:])
    v_sb = small.tile([B, JL * P], F32, tag="v")
    nc.scalar.copy(v_sb[:], v_ps[:])
    return v_sb


def build_kernel():
    if "nc" in _CACHED:
        return _CACHED["nc"]
    nc = bacc.Bacc(
        "TRN2", target_bir_lowering=False, debug=False, num_devices=N_CORES
    )
    wn_d = nc.dram_tensor("wn", [128, ICH * Q * JL * P], BF16, kind="ExternalInput")
    wt_d = nc.dram_tensor("wt", [128, I * Q], BF16, kind="ExternalInput")
    xq_d = nc.dram_tensor("xq", [128, ICH * Q * B], BF16, kind="ExternalInput")
    xt_d = nc.dram_tensor("xt", [128, I * Q], BF16, kind="ExternalInput")
    out_d = nc.dram_tensor("o", [B, JL * P], F32, kind="ExternalOutput")

    with tile.TileContext(nc) as tc:
        with (
            tc.tile_pool(name="big", bufs=1) as big,
            tc.tile_pool(name="wts", bufs=2) as wts,
            tc.tile_pool(name="evac", bufs=2) as evac,
            tc.tile_pool(name="ustr", bufs=2) as ustr,
            tc.tile_pool(name="small", bufs=1) as small,
            tc.tile_pool(name="ytile", bufs=2) as ytile,
            tc.tile_pool(name="dram", bufs=8, space="DRAM") as dram,
        ):
            # wt_s prefetch helper; first two windows fetched before the
            # bulk resident loads so iteration-0's b-pass can start early
            wt_tiles = {}

            def fetch_wt(it, g):
                t = wts.tile(
                    [128, 128 * Q], BF16, tag="wt_s", name=f"wt{it}_{g}"
                )
                nc.sync.dma_start(t[:], wt_d[:, g * 128 * Q:(g + 1) * 128 * Q])
                wt_tiles[(it, g)] = t

            fetch_wt(0, 0)
            fetch_wt(0, 1)

            # ---- resident loads (chunked by ich group) ---------------
            xq = big.tile([128, ICH * Q * B], BF16, tag="xq")        # 32K/part
            xqv = xq.rearrange("k (ich q b) -> k ich q b", ich=ICH, q=Q)
            wn = big.tile([128, ICH * Q * JL * P], BF16, tag="wn")   # 64K/part
            wnv = wn.rearrange("k (ich q j p) -> k ich q j p", ich=ICH, q=Q, j=JL)
            XQC = ICH * Q * B // 4
            WNC = ICH * Q * JL * P // 4
            for k in range(4):
                nc.sync.dma_start(
                    xq[:, k * XQC:(k + 1) * XQC], xq_d[:, k * XQC:(k + 1) * XQC]
                )
                nc.sync.dma_start(
                    wn[:, k * WNC:(k + 1) * WNC], wn_d[:, k * WNC:(k + 1) * WNC]
                )
            xt = big.tile([128, I * Q], BF16, tag="xt")              # 64K/part
            XTC = I * Q // 4
            for k in range(4):
                nc.sync.dma_start(
                    xt[:, k * XTC:(k + 1) * XTC],
                    xt_d[:, k * XTC:(k + 1) * XTC],
                )

            identity = big.tile([128, 128], F32, tag="ident")
            make_identity(nc, identity[:])
            identb = big.tile([128, 128], BF16, tag="identb")
            make_identity(nc, identb[:])
            eps_t = big.tile([128, 1], F32, tag="eps")
            nc.vector.memset(eps_t[:], EPS)

            # F[iw, (ich, j, b)] bf16: running c (up to per-(b,i) normalizer)
            f_sb = big.tile([128, ICH * JL * B], BF16, tag="f")      # 8K/part
            f_v = f_sb.rearrange("k (ich j b) -> k ich j b", ich=ICH, j=JL)

            # warmup collective to absorb core-start skew
            wu_s = small.tile([128, 8], F32, tag="wu")
            nc.gpsimd.memset(wu_s[:], 0.0)
            wu_i = dram.tile([128, 8], F32, tag="wu_i")
            wu_o = dram.tile([128, 8], F32, tag="wu_o")
            nc.gpsimd.dma_start(wu_i[:], wu_s[:])
            nc.gpsimd.collective_compute(
                "AllReduce", mybir.AluOpType.add,
                replica_groups=[list(range(N_CORES))],
                ins=[wu_i.opt()], outs=[wu_o.opt()],
            )

            # ---- S-pass: vT[(j,p), b] = sum_{i,q} W x ---------------
            with tc.tile_pool(name="ps_s", bufs=1, space="PSUM") as ps_s, \
                 tc.tile_pool(name="ps_st", bufs=1, space="PSUM") as ps_st:
                s_ps = ps_s.tile([128, B], F32, tag="s_ps")
                n_mm = ICH * Q
                k = 0
                for ich in range(ICH):
                    for q in range(Q):
                        nc.tensor.matmul(
                            s_ps[:],
                            wnv[:, ich, q, :, :],       # lhsT [128, (j p)]
                            xqv[:, ich, q, :],          # rhs  [128, 64]
                            start=(k == 0), stop=(k == n_mm - 1),
                        )
                        k += 1
                v_sb = _vT_to_v(nc, small, ps_st, s_ps, identity, scale=1.0 / J)
                _squash(nc, small, v_sb, eps_t)
                vbds = _build_vbd(nc, small, ps_st, v_sb, identity)

            # ---- 2 routing iterations -------------------------------
            for it in range(2):
                first = it == 0

                def v_chunk(ck, it=it):
                    """Emit v-pass work for AllReduce chunk ck."""
                    cl = CHUNKS[ck]
                    sl = slice(CH_OFF[ck], CH_OFF[ck + 1])
                    zh = small.tile(
                        [128, cl * B], BF16, tag=f"z{ck}",
                        name=f"z{it}_{ck}",
                    )
                    # trigger from Act: SP would stall the wt_s prefetch
                    # stream behind this AR-gated wait
                    nc.scalar.dma_start(zh[:], cc_out[ck][:])
                    zr = small.tile(
                        [128, cl * B], F32, tag=f"zr{ck}",
                        name=f"zr{it}_{ck}",
                    )
                    nc.vector.reciprocal(zr[:], zh[:])
                    zrv = zr.rearrange("k (ic b) -> k ic b", ic=cl)
                    zrb = zrv.unsqueeze(2).broadcast_to(
                        [128, cl, JL, B]
                    )
                    nc.gpsimd.tensor_mul(
                        f_v[:, sl, :, :], f_v[:, sl, :, :], zrb[:]
                    )
                    for ich in range(CH_OFF[ck], CH_OFF[ck + 1]):
                        for pr in range(2):
                            y = ytile.tile(
                                [128, 2 * Q * B], BF16, tag="y",
                                name=f"y{it}_{ich}_{pr}",
                            )
                            yv = y.rearrange(
                                "k (j q b) -> k j q b", j=2, q=Q
                            )
                            cb = (
                                f_v[:, ich, 2 * pr:2 * pr + 2, :]
                                .unsqueeze(2).broadcast_to([128, 2, Q, B])
                            )
                            xb = (
                                xqv[:, ich, :, :]
                                .unsqueeze(1).broadcast_to([128, 2, Q, B])
                            )
                            nc.vector.tensor_mul(yv[:], xb[:], cb[:])
                            for q in range(Q):
                                for jj in range(2):
                                    j = 2 * pr + jj
                                    nc.tensor.matmul(
                                        vT_ps[j * P:(j + 1) * P, :],
                                        wnv[:, ich, q, j, :],
                                        yv[:, jj, q, :],
                                        start=(ich == 0 and q == 0),
                                        stop=(ich == ICH - 1 and q == Q - 1),
                                        tile_position=(0, j * P),
                                    )

                # b-pass: wt cols per g are (iw 128, q 16); per half h the
                # 1024-col window is iw 64..(64h+63) x q 16.
                cc_out = [None] * NCHUNK
                with tc.tile_pool(name=f"ps_v{it}", bufs=1, space="PSUM") as ps_v:
                  vT_ps = ps_v.tile([128, B], F32, tag="vT_ps")
                  with tc.tile_pool(name=f"ps_b{it}", bufs=2, space="PSUM") as ps_b, \
                       tc.tile_pool(name=f"ps_bt{it}", bufs=2, space="PSUM") as ps_bt:
                    for g in range(ICH):
                        if (it, g) not in wt_tiles:
                            fetch_wt(it, g)
                        wt_s = wt_tiles.pop((it, g))
                        if g + 2 < ICH:
                            fetch_wt(it, g + 2)
                        for pair in range(2):
                            dwin = ustr.tile(
                                [128, 128], BF16, tag="dwin",
                                name=f"dwin{it}_{g}_{pair}",
                            )
                            for h in range(2):
                                t_ps = ps_b.tile(
                                    [128, 1024], F32, tag="t_ps",
                                    name=f"t_ps{it}_{g}_{pair}_{h}",
                                )
                                for m in range(2):
                                    nc.tensor.matmul(
                                        t_ps[:, m * 512:(m + 1) * 512],
                                        vbds[pair][:],
                                        wt_s[:, h * 1024 + m * 512:
                                             h * 1024 + (m + 1) * 512],
                                        start=True, stop=True,
                                    )
                                t_sb = evac.tile(
                                    [128, 1024], BF16, tag="t_sb",
                                    name=f"t_sb{it}_{g}_{pair}_{h}",
                                )
                                nc.scalar.copy(t_sb[:], t_ps[:])
                                u = ustr.tile(
                                    [128, 1024], BF16, tag="u",
                                    name=f"u{it}_{g}_{pair}_{h}",
                                )
                                xoff = g * 2048 + h * 1024
                                nc.vector.tensor_mul(
                                    u[:], t_sb[:], xt[:, xoff:xoff + 1024]
                                )
                                uq = u.rearrange("k (iw q) -> k iw q", q=Q)
                                with nc.allow_low_precision(
                                    reason="routing logits tolerate bf16"
                                ):
                                    nc.vector.tensor_reduce(
                                        dwin[:, h * 64:(h + 1) * 64],
                                        uq[:],
                                        mybir.AxisListType.X,
                                        mybir.AluOpType.add,
                                    )
                            d_ps = ps_bt.tile([128, 128], BF16, tag="d_ps")
                            nc.tensor.transpose(d_ps[:], dwin[:], identb[:])
                            off = (g * JL + pair * 2) * B
                            dst = f_sb[:, off:off + 2 * B]
                            if first:
                                nc.scalar.activation(dst, d_ps[:], AF.Exp)
                            else:
                                ex = ustr.tile([128, 128], BF16, tag="ex")
                                nc.scalar.activation(ex[:], d_ps[:], AF.Exp)
                                nc.gpsimd.tensor_mul(dst, dst, ex[:])
                        if g + 1 in CH_OFF:
                            ck = CH_OFF.index(g + 1) - 1
                            cl = CHUNKS[ck]
                            sl = slice(CH_OFF[ck], CH_OFF[ck + 1])
                            zph = small.tile(
                                [128, cl * B], BF16, tag=f"zp{ck}",
                                name=f"zp{it}_{ck}",
                            )
                            zpv = zph.rearrange("k (ic b) -> k ic b", ic=cl)
                            nc.gpsimd.tensor_add(
                                zpv[:], f_v[:, sl, 0, :], f_v[:, sl, 1, :]
                            )
                            for j in range(2, JL):
                                nc.gpsimd.tensor_add(
                                    zpv[:], zpv[:], f_v[:, sl, j, :]
                                )
                            cc_i = dram.tile(
                                [128, cl * B], BF16, tag=f"cc_i{ck}",
                                name=f"cci{it}_{ck}",
                            )
                            cc_o = dram.tile(
                                [128, cl * B], BF16, tag=f"cc_o{ck}",
                                name=f"cco{it}_{ck}", addr_space="Shared",
                            )
                            nc.gpsimd.dma_start(cc_i[:], zph[:])
                            nc.gpsimd.collective_compute(
                                "AllReduce", mybir.AluOpType.add,
                                replica_groups=[list(range(N_CORES))],
                                ins=[cc_i.opt()], outs=[cc_o.opt()],
                            )
                            cc_out[ck] = cc_o
                            if ck >= 2:
                                v_chunk(ck - 2)
                  # b-pass PSUM pools closed; finish the last two v-chunks
                  v_chunk(NCHUNK - 2)
                  v_chunk(NCHUNK - 1)
                  with tc.tile_pool(
                      name=f"ps_vt{it}", bufs=2, space="PSUM"
                  ) as ps_vt:
                    v_sb = _vT_to_v(nc, small, ps_vt, vT_ps, identity)
                    _squash(nc, small, v_sb, eps_t)
                    if it == 0:
                        vbds = _build_vbd(nc, small, ps_vt, v_sb, identity)
                    else:
                        nc.sync.dma_start(out_d[:], v_sb[:])

    nc.compile()
    _CACHED["nc"] = nc
    return nc


def _prep_inputs(inputs_np, W_np):
    x = np.ascontiguousarray(inputs_np)           # [B, I, Q] f32
    W = np.ascontiguousarray(W_np)                # [J, I, P, Q] f32
    xq = (
        x.reshape(B, ICH, 128, Q).transpose(2, 1, 3, 0)
        .astype(NP_BF16).reshape(128, ICH * Q * B)
    )
    # xt cols ordered (g, iw, q): natural x layout; b-duplication is on-chip
    xt_base = x.astype(NP_BF16).reshape(B, I * Q)
    xt = np.concatenate([xt_base, xt_base], axis=0)
    in_maps = []
    for r in range(N_CORES):
        Wr = W[r * JL:(r + 1) * JL]                       # [4, I, P, Q]
        wn = (
            Wr.reshape(JL, ICH, 128, P, Q).transpose(2, 1, 4, 0, 3)
            .astype(NP_BF16).reshape(128, ICH * Q * JL * P)
        )
        # wt rows (j,p); cols ordered (g, iw, q)
        wt = (
            Wr.reshape(JL, ICH, 128, P, Q)
            .transpose(0, 3, 1, 2, 4)                     # [j, p, g, iw, q]
            .astype(NP_BF16).reshape(128, I * Q)
        )
        in_maps.append(
            {
                "wn": np.ascontiguousarray(wn),
                "wt": np.ascontiguousarray(wt),
                "xq": np.ascontiguousarray(xq),
                "xt": np.ascontiguousarray(xt),
            }
        )
    return in_maps


def kernel(inputs, W, _trace=False):
    nc = build_kernel()
    in_maps = _prep_inputs(np.asarray(inputs), np.asarray(W))
    res = run_bass_kernel_spmd(nc, in_maps, list(range(N_CORES)), trace=_trace)
    out = np.concatenate(
        [res.results[r]["o"].reshape(B, JL, P) for r in range(N_CORES)], axis=1
    )
    if _trace:
        kernel.last_exec_ns = res.exec_time_ns
        kernel.last_results = res
    return out.astype(np.float32)
